# revision 6
# baseline (speedup 1.0000x reference)
"""Trainium2 Bass kernel for a full transformer block (LN->MHA->LN->FFN).

Sharding: 4-way data-parallel over batch x 2-way tensor-parallel heads for
attention; after the attention projection a pairwise ReduceScatter hands each
core half of the rows, and the FFN runs full-width on those T/2 rows (no
second collective, residual fused into the FFN epilogue).

Core c handles batch c//2 with head-group g=c%2; it owns rows
{512*t + 256*g .. 512*t + 256*(g+1)} for t in 0..3.

The program is software-pipelined over 512-row chunks: phase1 =
LN1+QKV+attention+proj -> per-chunk ReduceScatter; FFN super-chunks (512 own
rows each) are emitted between phase1 chunks so collective latency hides
under compute.

LayerNorm scale vectors are folded into Wq/Wk/Wv/W1 rows on the host.

Self-contained: hardcodes the full-problem shapes; builds per-core input
slices on the host, runs one SPMD Bass program on 8 NeuronCores.
"""

import os
import numpy as np
import ml_dtypes

import concourse.bacc as bacc
import concourse.tile as tile
from concourse import mybir
from concourse.bass_utils import run_bass_kernel_spmd

STUB_CC = os.environ.get("STUB_CC") == "1"  # replace collectives with DMA copies (timing experiments only)
F32 = mybir.dt.float32
F32R = mybir.dt.float32r
BF16 = mybir.dt.bfloat16
EPS = 1e-5


class Cfg:
    def __init__(self, B, T, E, HPC, FH, n_cores):
        self.B, self.T, self.E, self.HPC, self.FH = B, T, E, HPC, FH
        self.n_cores = n_cores
        self.HS = 64
        self.D = HPC * self.HS          # local head dims (= cols of Wq slice)
        self.PAIRS = HPC // 2           # 128-col head-pair groups
        self.TT = T // 128              # t-tiles
        self.QCW = min(512, T)          # q-chunk width for attention
        self.TC = T // self.QCW        # q/t-chunks
        self.KTPQ = self.QCW // 128     # k-tiles per q-chunk block
        self.TPC = self.QCW // 128      # t-tiles per chunk
        self.EC = E // 128              # e-chunks
        self.NH = min(512, E)           # matmul N for E-wide outputs
        self.EH = E // self.NH          # n-halves of E
        self.FC = FH // 128             # FFN hidden chunks (full 4E width)
        self.TO = T // 2                # rows owned after ReduceScatter
        self.SC = self.TO // self.QCW   # FFN super-chunks (512 own rows)
        self.scale = 1.0 / np.sqrt(E)


FULL = Cfg(B=4, T=2048, E=1024, HPC=8, FH=4096, n_cores=8)


def build_nc(cfg):
    c = cfg
    nc = bacc.Bacc(
        "TRN2", target_bir_lowering=False, debug=False, num_devices=c.n_cores
    )
    pairs_rg = [[2 * i, 2 * i + 1] for i in range(c.n_cores // 2)]

    # ---- DRAM I/O ----
    x_d = nc.dram_tensor("x", [c.T, c.E], F32, kind="ExternalInput")
    xo_d = nc.dram_tensor("xo", [c.TO, c.E], F32, kind="ExternalInput")
    wq_d = nc.dram_tensor("wq", [c.E, c.D], BF16, kind="ExternalInput")
    wk_d = nc.dram_tensor("wk", [c.E, c.D], BF16, kind="ExternalInput")
    wv_d = nc.dram_tensor("wv", [c.E, c.D], BF16, kind="ExternalInput")
    wo_d = nc.dram_tensor("wo", [c.D, c.E], BF16, kind="ExternalInput")
    w1_d = nc.dram_tensor("w1", [c.E, c.FH], BF16, kind="ExternalInput")
    w2_d = nc.dram_tensor("w2", [c.FH, c.E], BF16, kind="ExternalInput")
    b1_d = nc.dram_tensor("b1", [128, c.FC], F32, kind="ExternalInput")
    bo_d = nc.dram_tensor("bor", [128, c.E], F32, kind="ExternalInput")
    b2_d = nc.dram_tensor("b2r", [128, c.E], F32, kind="ExternalInput")
    msk_d = nc.dram_tensor(
        "masks", [128, 2 * c.KTPQ * c.QCW], BF16, kind="ExternalInput"
    )
    id_d = nc.dram_tensor("ident", [128, 128], F32R, kind="ExternalInput")
    out_d = nc.dram_tensor("out", [c.TO, c.E], F32, kind="ExternalOutput")

    # ---- persistent SBUF ----
    qkT = nc.alloc_sbuf_tensor("qkT", [128, 2 * c.PAIRS * c.T], BF16).ap()

    def qT(p):
        return qkT[:, p * c.T:(p + 1) * c.T]

    def kT(p):
        return qkT[:, (c.PAIRS + p) * c.T:(c.PAIRS + p + 1) * c.T]

    aT_sb = nc.alloc_sbuf_tensor("aT_sb", [128, c.FC * c.QCW], BF16).ap()

    def aT(f):
        return aT_sb[:, f * c.QCW:(f + 1) * c.QCW]

    v_sb = nc.alloc_sbuf_tensor("v_sb", [128, c.TT * c.HPC * 65], BF16).ap()

    def v_aug(tt, h):
        o = (tt * c.HPC + h) * 65
        return v_sb[:, o:o + 65]

    NSLOTW = 3
    attT = nc.alloc_sbuf_tensor("attT", [128, NSLOTW * 2 * c.QCW], BF16).ap()

    # x2 residual tiles for the FFN epilogue (one super-chunk in flight)
    x2_sb = nc.alloc_sbuf_tensor("x2_sb", [128, c.TPC * c.E], BF16).ap()

    def x2t(tt_loc):
        return x2_sb[:, tt_loc * c.E:(tt_loc + 1) * c.E]

    ident = nc.alloc_sbuf_tensor("ident_sb", [128, 128], F32R).ap()
    masks = nc.alloc_sbuf_tensor("masks_sb", [128, 2 * c.KTPQ * c.QCW], BF16).ap()
    bo_r = nc.alloc_sbuf_tensor("bo_sb", [128, c.E], F32).ap()
    b2_r = nc.alloc_sbuf_tensor("b2_sb", [128, c.E], F32).ap()
    b1_sb = nc.alloc_sbuf_tensor("b1_sb", [128, c.FC], F32).ap()
    eps_sb = nc.alloc_sbuf_tensor("eps_sb", [128, 1], F32).ap()

    # ---- internal DRAM ----
    ar1_in = nc.dram_tensor("ar1_in", [c.T, c.E], F32, kind="Internal")
    ar1_out = nc.dram_tensor("ar1_out", [c.TO, c.E], F32, kind="Internal")

    with tile.TileContext(nc) as tc:
        with (
            tc.tile_pool(name="io", bufs=4) as io,
            tc.tile_pool(name="hT", bufs=2) as hpool,
            tc.tile_pool(name="yTp", bufs=2) as ypool,
            tc.tile_pool(name="scr", bufs=1) as scr,
            tc.tile_pool(name="stat", bufs=2) as stat,
            tc.tile_pool(name="wqk", bufs=3) as wqk_pool,
            tc.tile_pool(name="w1p", bufs=2) as w1_pool,
            tc.tile_pool(name="w2p", bufs=3) as w2_pool,
            tc.tile_pool(name="wvp", bufs=1) as wv_pool,
            tc.tile_pool(name="wop", bufs=1) as wo_pool,
            tc.tile_pool(name="rcp", bufs=1) as rcp,
            tc.tile_pool(name="ps_w", bufs=2, space="PSUM") as ps_w,
            tc.tile_pool(name="ps_tp", bufs=1, space="PSUM") as ps_tp,
            tc.tile_pool(name="ps_acc", bufs=1, space="PSUM") as ps_acc,
            tc.tile_pool(name="ps_yps", bufs=1, space="PSUM") as ps_yps,
        ):
            # ---- consts ----
            nc.gpsimd.dma_start(ident[:], id_d[:])
            nc.gpsimd.dma_start(masks[:], msk_d[:])
            nc.gpsimd.dma_start(bo_r[:], bo_d[:])
            nc.gpsimd.dma_start(b2_r[:], b2_d[:])
            nc.gpsimd.dma_start(b1_sb[:], b1_d[:])
            nc.vector.memset(eps_sb[:], EPS)

            def layernorm_tile(xt):
                """xt: [128, E] f32 SBUF -> h [128, E] f32r tile.

                Scale weight is pre-folded into the consumer matmul weights.
                rsqrt(v) = exp(-0.5*ln(v)) keeps ACT on one table set
                (natural_log_exp_and_others: exp+ln+relu) for the kernel.
                """
                ng = c.E // 512
                bst = stat.tile([128, 6 * ng], F32, tag="bst")
                bst3 = bst[:].rearrange("p (g s) -> p g s", g=ng)
                for g in range(ng):
                    nc.vector.bn_stats(
                        bst3[:, g:g + 1, :],
                        xt[:, g * 512:(g + 1) * 512].rearrange(
                            "p (g w) -> p g w", g=1
                        ),
                    )
                mv = stat.tile([128, 2], F32, tag="mv")
                nc.vector.bn_aggr(
                    mv[:], bst[:].rearrange("p (g s) -> p g s", g=ng)
                )
                mu = mv[:, 0:1]
                lnv = stat.tile([128, 1], F32, tag="lnv")
                nc.scalar.activation(
                    lnv[:], mv[:, 1:2], mybir.ActivationFunctionType.Ln,
                    bias=eps_sb[:],
                )
                rsig = stat.tile([128, 1], F32, tag="rsig")
                nc.scalar.activation(
                    rsig[:], lnv[:], mybir.ActivationFunctionType.Exp,
                    scale=-0.5,
                )
                h = scr.tile([128, c.E], F32R, tag="h")
                nc.vector.tensor_scalar(
                    h[:], xt[:], mu, rsig[:],
                    mybir.AluOpType.subtract, mybir.AluOpType.mult,
                )
                return h

            TG = 4  # transposes per psum tile

            def transpose_to(h, hTc, tt_loc):
                """h [128,E] f32r -> hTc e-chunk columns tt_loc (transposed)."""
                dst3 = hTc.rearrange("p (e w) -> p e w", e=c.EC)[
                    :, :, tt_loc * 128:(tt_loc + 1) * 128
                ]
                for g0 in range(0, c.EC, TG):
                    tp = ps_tp.tile([128, TG * 128], F32R, tag="tp")
                    for i in range(TG):
                        e = g0 + i
                        nc.tensor.matmul(
                            tp[:, i * 128:(i + 1) * 128],
                            h[:, e * 128:(e + 1) * 128],
                            ident[:],
                            is_transpose=True, start=True, stop=True,
                        )
                    nc.vector.tensor_copy(
                        dst3[:, g0:g0 + TG, :],
                        tp[:].rearrange("p (g w) -> p g w", g=TG),
                    )

            slot_ctr = [0]

            def att_block(p, qc, yTc):
                """Attention for head pair p, q-chunk qc (kT/v ready).

                Even/odd head scores live in halves of one wide [128,1024]
                PSUM tile so exp and masking are single wide ops.
                """
                last = c.KTPQ * qc + c.KTPQ - 1
                q0 = qc * c.QCW
                W = c.QCW
                yps = ps_yps.tile([65, 2 * W], F32, tag="yps")
                pend = []

                def issue_av(kt, cq0, aw):
                    st, sp = kt == 0, kt == last
                    nc.tensor.matmul(
                        yps[:, cq0:W], v_aug(kt, 2 * p), aw[:, cq0:W],
                        start=st, stop=sp,
                    )
                    nc.tensor.matmul(
                        yps[:, W + cq0:], v_aug(kt, 2 * p + 1),
                        aw[:, W + cq0:],
                        start=st, stop=sp,
                    )

                for kt in range(last + 1):
                    j = kt - c.KTPQ * qc  # >=0: diagonal block stripe
                    # columns q < j*128 are fully masked: skip them entirely
                    cq0 = max(0, j) * 128
                    sw = ps_w.tile([128, 2 * W], F32, tag="w")
                    for hh in (0, 1):
                        off = hh * 64
                        nc.tensor.matmul(
                            sw[:, hh * W + cq0:(hh + 1) * W],
                            kT(p)[off:off + 64, kt * 128:(kt + 1) * 128],
                            qT(p)[off:off + 64, q0 + cq0:q0 + c.QCW],
                            start=True, stop=True,
                            tile_position=(off, 0),
                        )
                    s0 = (slot_ctr[0] % NSLOTW) * 2 * W
                    aw = attT[:, s0:s0 + 2 * W]
                    slot_ctr[0] += 1
                    # one exp over both halves (strided past skipped cols)
                    nc.scalar.activation(
                        aw.rearrange("p (h w) -> p h w", h=2)[:, :, cq0:],
                        sw[:].rearrange("p (h w) -> p h w", h=2)[:, :, cq0:],
                        mybir.ActivationFunctionType.Exp,
                    )
                    if j >= 0:  # triangular mask on the surviving stripe
                        m2 = masks[:, 2 * j * W:2 * (j + 1) * W].rearrange(
                            "p (h w) -> p h w", h=2
                        )[:, :, cq0:]
                        a3 = aw.rearrange("p (h w) -> p h w", h=2)[:, :, cq0:]
                        nc.vector.tensor_mul(a3, a3, m2)
                    pend.append((kt, cq0, aw))
                    if len(pend) > 1:
                        issue_av(*pend.pop(0))
                while pend:
                    issue_av(*pend.pop(0))

                # normalize: yTc[p] rows = yps[0:64] * (1/yps[64])
                rc = rcp.tile([1, 2 * W], F32, tag="rc")
                nc.vector.reciprocal(rc[:], yps[64:65, :])
                rb = rcp.tile([64, 2 * W], F32, tag="rb")
                nc.gpsimd.partition_broadcast(rb[:], rc[:])
                for hh in (0, 1):
                    nc.vector.tensor_mul(
                        yTc[hh * 64:hh * 64 + 64, p * W:(p + 1) * W],
                        rb[:, hh * W:(hh + 1) * W],
                        yps[0:64, hh * W:(hh + 1) * W],
                    )

            def phase1a(tcc):
                """LN1 + V + QK for chunk tcc; returns wot for phase1b."""
                r0 = tcc * c.QCW  # first row of chunk
                hTc = hpool.tile([128, c.EC * c.QCW], BF16, tag="hT")

                # x tiles first so LN can start before weight DMAs queue
                xts = []
                for tt_loc in range(c.TPC):
                    tt = tcc * c.TPC + tt_loc
                    xt = io.tile([128, c.E], F32, tag="io")
                    nc.sync.dma_start(xt[:], x_d[tt * 128:(tt + 1) * 128, :])
                    xts.append(xt)

                # weight prefetch for this chunk
                wvt = wv_pool.tile([128, c.EC * c.D], BF16, tag="wv")
                nc.sync.dma_start(
                    wvt[:].rearrange("p (e d) -> p e d", e=c.EC),
                    wv_d[:].rearrange("(e p) d -> p e d", p=128),
                )
                wot = wo_pool.tile([128, c.PAIRS * c.E], BF16, tag="wo")
                nc.sync.dma_start(
                    wot[:].rearrange("p (d e) -> p d e", d=c.PAIRS),
                    wo_d[:].rearrange("(d p) e -> p d e", p=128),
                )

                for tt_loc in range(c.TPC):
                    h = layernorm_tile(xts[tt_loc])
                    transpose_to(h, hTc[:, :], tt_loc)

                # V for this chunk's t-tiles
                for tt_loc in range(c.TPC):
                    tt = tcc * c.TPC + tt_loc
                    vps = ps_acc.tile([128, c.D], F32, tag="acc")
                    for e in range(c.EC):
                        nc.tensor.matmul(
                            vps[:],
                            hTc[:, e * c.QCW + tt_loc * 128:][:, :128],
                            wvt[:, e * c.D:(e + 1) * c.D],
                            start=(e == 0), stop=(e == c.EC - 1),
                        )
                    vdst = v_sb[
                        :, tt * c.HPC * 65:(tt + 1) * c.HPC * 65
                    ].rearrange("p (h w) -> p h w", w=65)
                    nc.vector.tensor_copy(
                        vdst[:, :, 0:64],
                        vps[:].rearrange("p (h w) -> p h w", w=64),
                    )
                    nc.vector.memset(vdst[:, :, 64:65], 1.0)

                # Q/K for this chunk
                for p in range(c.PAIRS):
                    wqt = wqk_pool.tile([128, c.E], BF16, tag="wqk")
                    nc.sync.dma_start(
                        wqt[:].rearrange("p (e m) -> p e m", e=c.EC),
                        wq_d[:, p * 128:(p + 1) * 128].rearrange(
                            "(e p) m -> p e m", p=128
                        ),
                    )
                    wkt = wqk_pool.tile([128, c.E], BF16, tag="wqk")
                    nc.sync.dma_start(
                        wkt[:].rearrange("p (e m) -> p e m", e=c.EC),
                        wk_d[:, p * 128:(p + 1) * 128].rearrange(
                            "(e p) m -> p e m", p=128
                        ),
                    )
                    qk = ps_w.tile([128, 2 * c.QCW], F32, tag="w")
                    for half, wt in ((0, wqt), (1, wkt)):
                        for e in range(c.EC):
                            nc.tensor.matmul(
                                qk[:, half * c.QCW:(half + 1) * c.QCW],
                                wt[:, e * 128:(e + 1) * 128],
                                hTc[:, e * c.QCW:(e + 1) * c.QCW],
                                start=(e == 0), stop=(e == c.EC - 1),
                            )
                    # q slice and k slice of qkT are PAIRS*T cols apart
                    dqk = qkT.rearrange(
                        "p (s w) -> p s w", w=c.PAIRS * c.T
                    )[:, :, p * c.T + r0:p * c.T + r0 + c.QCW]
                    nc.vector.tensor_copy(
                        dqk, qk[:].rearrange("p (s w) -> p s w", s=2)
                    )

                return wot

            def phase1b(tcc, wot):
                """Attention + proj -> ar1_in chunk + ReduceScatter."""
                r0 = tcc * c.QCW
                yTc = ypool.tile([128, c.PAIRS * c.QCW], BF16, tag="yT")
                for p in range(c.PAIRS):
                    att_block(p, tcc, yTc[:, :])

                for tt_loc in range(c.TPC):
                    tt = tcc * c.TPC + tt_loc
                    pt = io.tile([128, c.E], F32, tag="io")
                    pp = ps_w.tile([128, c.EH * c.NH], F32, tag="w")
                    for eh in range(c.EH):
                        for d in range(c.PAIRS):
                            nc.tensor.matmul(
                                pp[:, eh * c.NH:(eh + 1) * c.NH],
                                yTc[:, d * c.QCW + tt_loc * 128:][:, :128],
                                wot[:, d * c.E + eh * c.NH:][:, :c.NH],
                                start=(d == 0), stop=(d == c.PAIRS - 1),
                            )
                    nc.vector.tensor_copy(pt[:], pp[:])
                    nc.sync.dma_start(ar1_in[tt * 128:(tt + 1) * 128, :], pt[:])

                rows = slice(r0, r0 + c.QCW)
                orows = slice(tcc * 256, (tcc + 1) * 256)
                if c.n_cores == 1 or STUB_CC:  # timeline/profiling variant
                    nc.sync.dma_start(ar1_out[orows, :], ar1_in[rows, :][0:256, :])
                else:
                    nc.gpsimd.collective_compute(
                        "ReduceScatter", mybir.AluOpType.add,
                        replica_groups=pairs_rg,
                        ins=[ar1_in[rows, :]], outs=[ar1_out[orows, :]],
                    )

            def ffn_a(sc):
                """Own-row chunk sc: x2 = xo + rs + bo; LN2 -> transposed h2."""
                hTc = hpool.tile([128, c.EC * c.QCW], BF16, tag="hT")

                for tt_loc in range(c.TPC):
                    ro = sc * c.QCW + tt_loc * 128
                    xt = io.tile([128, c.E], F32, tag="io")
                    nc.sync.dma_start(xt[:], xo_d[ro:ro + 128, :])
                    at = io.tile([128, c.E], F32, tag="io")
                    nc.sync.dma_start(at[:], ar1_out[ro:ro + 128, :])
                    x2 = io.tile([128, c.E], F32, tag="io")
                    nc.vector.tensor_add(x2[:], xt[:], at[:])
                    nc.vector.tensor_add(x2[:], x2[:], bo_r[:])
                    nc.vector.tensor_copy(x2t(tt_loc), x2[:])
                    h2 = layernorm_tile(x2)
                    transpose_to(h2, hTc[:, :], tt_loc)
                return hTc

            def w1_load(fg):
                w1t = w1_pool.tile([128, 2 * c.E], BF16, tag="w1")
                nc.sync.dma_start(
                    w1t[:].rearrange("p (e m) -> p e m", e=c.EC),
                    w1_d[:, 2 * fg * 128:(2 * fg + 2) * 128].rearrange(
                        "(e p) m -> p e m", p=128
                    ),
                )
                return w1t

            def ffn_b(sc, hTc, w1pre=None):
                """Full-width FFN on own-row chunk sc; fused residual out."""
                def w2_load(eh, fg):
                    t = w2_pool.tile([128, 2 * c.NH], BF16, tag="w2")
                    nc.scalar.dma_start(
                        t[:].rearrange("p (g n) -> p g n", g=2),
                        w2_d[
                            2 * fg * 128:(2 * fg + 2) * 128,
                            eh * c.NH:(eh + 1) * c.NH,
                        ].rearrange("(g p) n -> p g n", p=128),
                    )
                    return t

                # FFN layer 1: aT[f] = relu(w1_f.T @ h2T + b1_f)
                # (prefetch first ffn2 w2 tiles under the tail of this loop)
                w2_pre = {}
                nfg = c.FC // 2
                for fg in range(nfg):
                    if w1pre is not None and fg == 0:
                        w1t = w1pre
                    else:
                        w1t = w1_load(fg)
                    if fg >= nfg - 2:
                        k = fg - (nfg - 2)
                        w2_pre[(0, k)] = w2_load(0, k)
                    for gi in range(2):
                        f = 2 * fg + gi
                        ap_ = ps_acc.tile([128, c.QCW], F32, tag="acc")
                        for e in range(c.EC):
                            nc.tensor.matmul(
                                ap_[:],
                                w1t[:, e * 256 + gi * 128:][:, :128],
                                hTc[:, e * c.QCW:(e + 1) * c.QCW],
                                start=(e == 0), stop=(e == c.EC - 1),
                            )
                        nc.scalar.activation(
                            aT(f), ap_[:], mybir.ActivationFunctionType.Relu,
                            bias=b1_sb[:, f:f + 1],
                        )

                # FFN layer 2 + residual epilogue: out rows = x2 + ff + b2
                for eh in range(c.EH):
                    ffps = []
                    for _fi in range(c.TPC // 2):
                        fftile = ps_w.tile([128, 2 * c.NH], F32, tag="w")
                        ffps.append(fftile)
                    for fg in range(nfg):
                        w2t = w2_pre.pop((eh, fg), None)
                        if w2t is None:
                            w2t = w2_load(eh, fg)
                        for gi in range(2):
                            f = 2 * fg + gi
                            for ti in range(c.TPC):
                                nc.tensor.matmul(
                                    ffps[ti // 2][
                                        :,
                                        (ti % 2) * c.NH:
                                        (ti % 2 + 1) * c.NH,
                                    ],
                                    aT(f)[
                                        :,
                                        ti * 128:(ti + 1) * 128,
                                    ],
                                    w2t[:, gi * c.NH:(gi + 1) * c.NH],
                                    start=(f == 0), stop=(f == c.FC - 1),
                                )
                    for wi in range(c.TPC // 2):
                        for bi in range(2):
                            ti = 2 * wi + bi
                            ro = sc * c.QCW + ti * 128
                            ft = io.tile([128, c.NH], F32, tag="ffout")
                            nc.vector.tensor_add(
                                ft[:],
                                ffps[wi][:, bi * c.NH:(bi + 1) * c.NH],
                                x2t(ti)[:, eh * c.NH:(eh + 1) * c.NH],
                            )
                            nc.vector.tensor_add(
                                ft[:], ft[:],
                                b2_r[:, eh * c.NH:(eh + 1) * c.NH],
                            )
                            nc.sync.dma_start(
                                out_d[ro:ro + 128, eh * c.NH:(eh + 1) * c.NH],
                                ft[:],
                            )

            # ---- software-pipelined emission over chunks ----
            # RS(tcc) feeds ffn super-chunk tcc//2 (needs RS 2s and 2s+1).
            wot = phase1a(0)
            phase1b(0, wot)
            wot = phase1a(1)
            phase1b(1, wot)
            wot = phase1a(2)
            h2a = ffn_a(0)
            phase1b(2, wot)
            w1pre = w1_load(0)
            wot = phase1a(3)
            ffn_b(0, h2a, w1pre=w1pre)
            phase1b(3, wot)
            h2b = ffn_a(1)
            ffn_b(1, h2b)

    nc.compile()
    return nc


def make_masks(cfg):
    c = cfg
    m = np.zeros((128, 2 * c.KTPQ * c.QCW), dtype=np.float32)
    for j in range(c.KTPQ):
        k = np.arange(128)[:, None]
        q = np.arange(c.QCW)[None, :]
        mj = (j * 128 + k <= q).astype(np.float32)
        m[:, 2 * j * c.QCW:(2 * j + 1) * c.QCW] = mj
        m[:, (2 * j + 1) * c.QCW:(2 * j + 2) * c.QCW] = mj
    return np.ascontiguousarray(m.astype(ml_dtypes.bfloat16))


def own_rows(cfg, g):
    """Global row indices owned by group-half g after the ReduceScatter."""
    c = cfg
    idx = []
    for tcc in range(c.TC):
        s = tcc * c.QCW + g * 256
        idx.extend(range(s, s + 256))
    return np.array(idx)


def make_in_maps(cfg, inputs):
    """Build the per-core input dicts from the full problem inputs."""
    c = cfg
    x = np.asarray(inputs["x"], dtype=np.float32)
    ln1 = np.asarray(inputs["ln1_w"], dtype=np.float32)
    ln2 = np.asarray(inputs["ln2_w"], dtype=np.float32)
    # fold LN scale vectors into the consumer weight rows
    Wq = (ln1[:, None] * np.asarray(inputs["Wq"], dtype=np.float32)
          * (1.0 / np.sqrt(c.E)))
    Wk = ln1[:, None] * np.asarray(inputs["Wk"], dtype=np.float32)
    Wv = ln1[:, None] * np.asarray(inputs["Wv"], dtype=np.float32)
    W1 = ln2[:, None] * np.asarray(inputs["W1"], dtype=np.float32)
    Wo = np.asarray(inputs["Wo"], dtype=np.float32)
    W2 = np.asarray(inputs["W2"], dtype=np.float32)
    bo = np.asarray(inputs["bo"], dtype=np.float32)
    b1 = np.asarray(inputs["b1"], dtype=np.float32)
    b2 = np.asarray(inputs["b2"], dtype=np.float32)

    def rep(v):
        return np.ascontiguousarray(
            np.broadcast_to(v[None, :], (128, c.E)).astype(np.float32)
        )

    consts = {
        "bor": rep(bo), "b2r": rep(b2),
        "masks": make_masks(c),
        "ident": np.eye(128, dtype=np.float32),
        "w1": np.ascontiguousarray(W1.astype(ml_dtypes.bfloat16)),
        "w2": np.ascontiguousarray(W2.astype(ml_dtypes.bfloat16)),
        "b1": np.ascontiguousarray(b1.reshape(c.FC, 128).T),
    }
    in_maps = []
    for core in range(c.n_cores):
        b, g = core // 2, core % 2
        d0, d1 = g * c.D, (g + 1) * c.D
        m = {
            "x": np.ascontiguousarray(x[b]),
            "xo": np.ascontiguousarray(x[b][own_rows(c, g)]),
            "wq": np.ascontiguousarray(Wq[:, d0:d1].astype(ml_dtypes.bfloat16)),
            "wk": np.ascontiguousarray(Wk[:, d0:d1].astype(ml_dtypes.bfloat16)),
            "wv": np.ascontiguousarray(Wv[:, d0:d1].astype(ml_dtypes.bfloat16)),
            "wo": np.ascontiguousarray(Wo[d0:d1, :].astype(ml_dtypes.bfloat16)),
        }
        m.update(consts)
        in_maps.append(m)
    return in_maps


_NC_CACHE = {}


def get_nc(cfg):
    key = (cfg.B, cfg.T, cfg.E, cfg.HPC, cfg.FH, cfg.n_cores)
    if key not in _NC_CACHE:
        _NC_CACHE[key] = build_nc(cfg)
    return _NC_CACHE[key]


def assemble_out(cfg, results):
    c = cfg
    out = np.empty((c.B, c.T, c.E), dtype=np.float32)
    for b in range(c.B):
        for g in range(2):
            out[b][own_rows(c, g)] = results[2 * b + g]["out"]
    return out


def kernel(**inputs) -> np.ndarray:
    c = FULL
    nc = get_nc(c)
    in_maps = make_in_maps(c, inputs)
    res = run_bass_kernel_spmd(nc, in_maps, core_ids=list(range(c.n_cores)))
    return assemble_out(c, res.results)


# revision 115
# speedup vs baseline: 3.1937x; 3.1937x over previous
"""Trainium2 Bass kernel for a full transformer block (LN->MHA->LN->FFN).

Sharding: 4-way data-parallel over batch x 2-way tensor-parallel heads for
attention; after the attention projection a pairwise ReduceScatter hands each
core half of the rows, and the FFN runs full-width on those T/2 rows (no
second collective, residual fused into the FFN epilogue).

Core c handles batch c//2 with head-group g=c%2; it owns rows
{512*t + 256*g .. 512*t + 256*(g+1)} for t in 0..3.

The program is software-pipelined over 512-row chunks: phase1 =
LN1+QKV+attention+proj -> per-chunk ReduceScatter; FFN super-chunks (512 own
rows each) are emitted between phase1 chunks so collective latency hides
under compute.

LayerNorm scale vectors are folded into Wq/Wk/Wv/W1 rows on the host.

Self-contained: hardcodes the full-problem shapes; builds per-core input
slices on the host, runs one SPMD Bass program on 8 NeuronCores.
"""

import os
import numpy as np
import ml_dtypes

import concourse.bacc as bacc
import concourse.tile as tile
from concourse import mybir
from concourse.bass_utils import run_bass_kernel_spmd

STUB_CC = os.environ.get("STUB_CC") == "1"  # replace collectives with DMA copies (timing experiments only)
# scheduler hints: don't consider the RS-dependent LN2 chains ready before
# these (ms) timestamps, so they can't clog engine queues ahead of
# attention-critical work
FFN_A0_MS = float(os.environ.get("FFN_A0_MS", "0.15"))
FFN_A1_MS = float(os.environ.get("FFN_A1_MS", "0.29"))
FFN_A1B_MS = float(os.environ.get("FFN_A1B_MS", "0.45"))
F32 = mybir.dt.float32
F32R = mybir.dt.float32r
BF16 = mybir.dt.bfloat16
EPS = 1e-5


class Cfg:
    def __init__(self, B, T, E, HPC, FH, n_cores):
        self.B, self.T, self.E, self.HPC, self.FH = B, T, E, HPC, FH
        self.n_cores = n_cores
        self.HS = 64
        self.D = HPC * self.HS          # local head dims (= cols of Wq slice)
        self.PAIRS = HPC // 2           # 128-col head-pair groups
        self.TT = T // 128              # t-tiles
        self.QCW = min(512, T)          # q-chunk width for attention
        self.TC = T // self.QCW        # q/t-chunks
        self.KTPQ = self.QCW // 128     # k-tiles per q-chunk block
        self.TPC = self.QCW // 128      # t-tiles per chunk
        self.EC = E // 128              # e-chunks
        self.NH = min(512, E)           # matmul N for E-wide outputs
        self.EH = E // self.NH          # n-halves of E
        self.FC = FH // 128             # FFN hidden chunks (full 4E width)
        self.TO = T // 2                # rows owned after ReduceScatter
        self.SC = self.TO // self.QCW   # FFN super-chunks (512 own rows)
        self.scale = 1.0 / np.sqrt(E)


FULL = Cfg(B=4, T=2048, E=1024, HPC=8, FH=4096, n_cores=8)


_real_act_tables = bacc.get_activation_tables


def _superset_act_tables(arch):
    """Hide Ln/Exp from every set except the one containing all our funcs.

    The table-load placement pass picks the first set containing each
    activation's func; with Ln and Exp visible only in
    natural_log_exp_and_others (which really does contain Ln+Exp+Relu+Copy),
    it loads that one set once instead of thrashing natural_log <->
    exp_and_others on every LayerNorm (48 x 1283 ns). Set ids/order are
    unchanged, so the emitted act_func_set_id stays valid on hardware.
    """
    A = mybir.ActivationFunctionType
    out = {}
    for name, s in _real_act_tables(arch).items():
        if name != "natural_log_exp_and_others":
            s = s - {A.Ln, A.Exp}
        out[name] = s
    return out


def build_nc(cfg):
    c = cfg
    bacc.get_activation_tables = _superset_act_tables
    try:
        return _build_nc(cfg)
    finally:
        bacc.get_activation_tables = _real_act_tables


def _build_nc(cfg):
    c = cfg
    nc = bacc.Bacc(
        "TRN2", target_bir_lowering=False, debug=False, num_devices=c.n_cores
    )
    pairs_rg = [[2 * i, 2 * i + 1] for i in range(c.n_cores // 2)]

    # ---- DRAM I/O ----
    x_d = nc.dram_tensor("x", [c.T, c.E], BF16, kind="ExternalInput")
    xo_d = nc.dram_tensor("xo", [c.TO, c.E], F32, kind="ExternalInput")
    wq_d = nc.dram_tensor("wq", [c.E, c.D], BF16, kind="ExternalInput")
    wk_d = nc.dram_tensor("wk", [c.E, c.D], BF16, kind="ExternalInput")
    wv_d = nc.dram_tensor("wv", [c.E, c.D], BF16, kind="ExternalInput")
    wo_d = nc.dram_tensor("wo", [c.D, c.E], BF16, kind="ExternalInput")
    w1_d = nc.dram_tensor("w1", [c.E, c.FH], BF16, kind="ExternalInput")
    w2_d = nc.dram_tensor("w2", [c.FH, c.E], BF16, kind="ExternalInput")
    b1_d = nc.dram_tensor("b1", [128, c.FC], F32, kind="ExternalInput")
    bo_d = nc.dram_tensor("bor", [128, c.E], BF16, kind="ExternalInput")
    b2_d = nc.dram_tensor("b2r", [128, c.E], BF16, kind="ExternalInput")
    msk_d = nc.dram_tensor(
        "masks", [128, c.KTPQ * c.QCW], BF16, kind="ExternalInput"
    )
    id_d = nc.dram_tensor("ident", [128, 128], F32R, kind="ExternalInput")
    idb_d = nc.dram_tensor("identb", [128, 128], BF16, kind="ExternalInput")
    out_d = nc.dram_tensor("out", [c.TO, c.E], F32, kind="ExternalOutput")

    # ---- persistent SBUF ----
    kT_sb = nc.alloc_sbuf_tensor("kT_sb", [128, c.PAIRS * c.T], BF16).ap()

    def kT(p):
        return kT_sb[:, p * c.T:(p + 1) * c.T]

    aT_sb = nc.alloc_sbuf_tensor("aT_sb", [128, c.FC * c.QCW], BF16).ap()

    def aT(f):
        return aT_sb[:, f * c.QCW:(f + 1) * c.QCW]

    v_sb = nc.alloc_sbuf_tensor("v_sb", [128, c.TT * c.HPC * 65], BF16).ap()

    def v_aug(tt, h):
        o = (tt * c.HPC + h) * 65
        return v_sb[:, o:o + 65]

    NSLOTW = 3
    attT = nc.alloc_sbuf_tensor("attT", [128, NSLOTW * 2 * c.QCW], BF16).ap()

    # x2 residual tiles for the FFN epilogue (one super-chunk per buffer;
    # sc1's buffer is a wv_pool slot reused after the last V matmul)
    x2_sb = nc.alloc_sbuf_tensor("x2_sb", [128, c.TPC * c.E], BF16).ap()

    ident = nc.alloc_sbuf_tensor("ident_sb", [128, 128], F32R).ap()
    identb = nc.alloc_sbuf_tensor("identb_sb", [128, 128], BF16).ap()
    masks = nc.alloc_sbuf_tensor("masks_sb", [128, c.KTPQ * c.QCW], BF16).ap()
    bo_r = nc.alloc_sbuf_tensor("bo_sb", [128, c.E], BF16).ap()
    b2_r = nc.alloc_sbuf_tensor("b2_sb", [128, c.E], BF16).ap()
    b1_sb = nc.alloc_sbuf_tensor("b1_sb", [128, c.FC], F32).ap()
    eps_sb = nc.alloc_sbuf_tensor("eps_sb", [128, 1], F32).ap()

    # ---- internal DRAM ----
    ar1_in = nc.dram_tensor("ar1_in", [c.T, c.E], BF16, kind="Internal")
    ar1_out = nc.dram_tensor("ar1_out", [c.TO, c.E], BF16, kind="Internal")

    with tile.TileContext(nc) as tc:
        with (
            tc.tile_pool(name="io", bufs=4) as io,
            tc.tile_pool(name="pout", bufs=2) as pout,
            tc.tile_pool(name="ffo", bufs=2) as ffo,
            tc.tile_pool(name="qp", bufs=2) as qpool,
            tc.tile_pool(name="hT", bufs=2) as hpool,
            tc.tile_pool(name="yTp", bufs=2) as ypool,
            tc.tile_pool(name="scr", bufs=2) as scr,
            tc.tile_pool(name="stat", bufs=2) as stat,
            tc.tile_pool(name="wqk", bufs=3) as wqk_pool,
            tc.tile_pool(name="w1p", bufs=2) as w1_pool,
            tc.tile_pool(name="w2p", bufs=3) as w2_pool,
            tc.tile_pool(name="wvp", bufs=1) as wv_pool,
            tc.tile_pool(name="wop", bufs=1) as wo_pool,
            tc.tile_pool(name="rcp", bufs=1) as rcp,
            tc.tile_pool(name="ps_w", bufs=2, space="PSUM") as ps_w,
            tc.tile_pool(name="ps_acc", bufs=2, space="PSUM") as ps_acc,
            tc.tile_pool(name="ps_yps", bufs=1, space="PSUM") as ps_yps,
        ):
            # ---- consts ----
            nc.gpsimd.dma_start(ident[:], id_d[:])
            nc.gpsimd.dma_start(identb[:], idb_d[:])
            nc.gpsimd.dma_start(masks[:], msk_d[:])
            nc.gpsimd.dma_start(bo_r[:], bo_d[:])
            nc.gpsimd.dma_start(b2_r[:], b2_d[:])
            nc.gpsimd.dma_start(b1_sb[:], b1_d[:])
            nc.vector.memset(eps_sb[:], EPS)

            def layernorm_tile(xt):
                """xt: [128, E] f32 SBUF -> h [128, E] f32r tile.

                Scale weight is pre-folded into the consumer matmul weights.
                rsqrt(v) = exp(-0.5*ln(v)) keeps ACT on one table set
                (natural_log_exp_and_others: exp+ln+relu) for the kernel.
                """
                ng = c.E // 512
                bst = stat.tile([128, 6 * ng], F32, tag="bst")
                bst3 = bst[:].rearrange("p (g s) -> p g s", g=ng)
                for g in range(ng):
                    nc.vector.bn_stats(
                        bst3[:, g:g + 1, :],
                        xt[:, g * 512:(g + 1) * 512].rearrange(
                            "p (g w) -> p g w", g=1
                        ),
                    )
                mv = stat.tile([128, 2], F32, tag="mv")
                nc.vector.bn_aggr(
                    mv[:], bst[:].rearrange("p (g s) -> p g s", g=ng)
                )
                mu = mv[:, 0:1]
                lnv = stat.tile([128, 1], F32, tag="lnv")
                nc.scalar.activation(
                    lnv[:], mv[:, 1:2], mybir.ActivationFunctionType.Ln,
                    bias=eps_sb[:],
                )
                rsig = stat.tile([128, 1], F32, tag="rsig")
                nc.scalar.activation(
                    rsig[:], lnv[:], mybir.ActivationFunctionType.Exp,
                    scale=-0.5,
                )
                h = scr.tile([128, c.E], F32R, tag="h")
                nc.vector.tensor_scalar(
                    h[:], xt[:], mu, rsig[:],
                    mybir.AluOpType.subtract, mybir.AluOpType.mult,
                )
                return h

            TG = 4  # transposes per psum tile

            def transpose_to(h, hTc, tt_loc):
                """h [128,E] f32r -> hTc e-chunk columns tt_loc (transposed)."""
                dst3 = hTc.rearrange("p (e w) -> p e w", e=c.EC)[
                    :, :, tt_loc * 128:(tt_loc + 1) * 128
                ]
                for g0 in range(0, c.EC, TG):
                    tp = ps_acc.tile([128, TG * 128], F32R, tag="acc")
                    for i in range(TG):
                        e = g0 + i
                        nc.tensor.matmul(
                            tp[:, i * 128:(i + 1) * 128],
                            h[:, e * 128:(e + 1) * 128],
                            ident[:],
                            is_transpose=True, start=True, stop=True,
                        )
                    nc.vector.tensor_copy(
                        dst3[:, g0:g0 + TG, :],
                        tp[:].rearrange("p (g w) -> p g w", g=TG),
                    )

            slot_ctr = [0]

            def att_block(p, qc, yTc, qt):
                """Attention for head pair p, q-chunk qc (kT/v ready).

                Even/odd head scores live in halves of one wide [128,1024]
                PSUM tile so exp and masking are single wide ops.
                """
                W = c.QCW
                yps = ps_yps.tile([65, 2 * W], F32, tag="yps")
                pend = []

                # descending stripe order: the diagonal stripe only needs
                # this chunk's own K/V, so attention can start before earlier
                # chunks' QKV are done
                kt_order = [
                    kt
                    for s in range(qc, -1, -1)
                    for kt in range(c.KTPQ * s, c.KTPQ * s + c.KTPQ)
                ]
                nkt = len(kt_order)

                def issue_av(idx, kt, cq0, aw):
                    st, sp = idx == 0, idx == nkt - 1
                    nc.tensor.matmul(
                        yps[:, cq0:W], v_aug(kt, 2 * p), aw[:, cq0:W],
                        start=st, stop=sp,
                    )
                    nc.tensor.matmul(
                        yps[:, W + cq0:], v_aug(kt, 2 * p + 1),
                        aw[:, W + cq0:],
                        start=st, stop=sp,
                    )

                for idx, kt in enumerate(kt_order):
                    j = kt - c.KTPQ * qc  # >=0: diagonal block stripe
                    # columns q < j*128 are fully masked: skip them entirely
                    cq0 = max(0, j) * 128
                    sw = ps_w.tile([128, 2 * W], F32, tag="w")
                    diag = j >= 0
                    for hh in (0, 1):
                        off = hh * 64
                        nc.tensor.matmul(
                            sw[:, hh * W + cq0:(hh + 1) * W],
                            kT(p)[off:off + 64, kt * 128:(kt + 1) * 128],
                            qt[off:off + 64, p * W + cq0:(p + 1) * W],
                            start=True, stop=not diag,
                            tile_position=(off, 0),
                        )
                    if diag:
                        # fold the causal mask in on the tensor engine:
                        # identity matmul accumulates -30 onto masked
                        # entries, so exp gives ~1e-13 ~= 0 (no DVE hop)
                        for hh in (0, 1):
                            nc.tensor.matmul(
                                sw[:, hh * W + cq0:(hh + 1) * W],
                                identb[:],
                                masks[:, j * W + cq0:(j + 1) * W],
                                start=False, stop=True,
                            )
                    s0 = (slot_ctr[0] % NSLOTW) * 2 * W
                    aw = attT[:, s0:s0 + 2 * W]
                    slot_ctr[0] += 1
                    # one exp over both halves (strided past skipped cols)
                    nc.scalar.activation(
                        aw.rearrange("p (h w) -> p h w", h=2)[:, :, cq0:],
                        sw[:].rearrange("p (h w) -> p h w", h=2)[:, :, cq0:],
                        mybir.ActivationFunctionType.Exp,
                    )
                    pend.append((idx, kt, cq0, aw))
                    if len(pend) > 1:
                        issue_av(*pend.pop(0))
                while pend:
                    issue_av(*pend.pop(0))

                # normalize: yTc[p] rows = yps[0:64] * (1/yps[64])
                rc = rcp.tile([1, 2 * W], F32, tag="rc")
                nc.vector.reciprocal(rc[:], yps[64:65, :])
                rb = rcp.tile([64, 2 * W], F32, tag="rb")
                nc.gpsimd.partition_broadcast(rb[:], rc[:])
                for hh in (0, 1):
                    nc.vector.tensor_mul(
                        yTc[hh * 64:hh * 64 + 64, p * W:(p + 1) * W],
                        rb[:, hh * W:(hh + 1) * W],
                        yps[0:64, hh * W:(hh + 1) * W],
                    )

            def load_wv_wo():
                wvt = wv_pool.tile([128, c.EC * c.D], BF16, tag="wv")
                nc.gpsimd.dma_start(
                    wvt[:].rearrange("p (e d) -> p e d", e=c.EC),
                    wv_d[:].rearrange("(e p) d -> p e d", p=128),
                )
                wot = wo_pool.tile([128, c.PAIRS * c.E], BF16, tag="wo")
                with tc.tile_wait_until(0.04):
                    nc.gpsimd.dma_start(
                        wot[:].rearrange("p (d e) -> p d e", d=c.PAIRS),
                        wo_d[:].rearrange("(d p) e -> p d e", p=128),
                    )
                return wvt, wot

            def phase1a(tcc, wvt):
                """LN1 + V + QK for chunk tcc."""
                r0 = tcc * c.QCW  # first row of chunk
                hTc = hpool.tile([128, c.EC * c.QCW], BF16, tag="hT")

                # x tiles first so LN can start before weight DMAs queue
                xts = []
                for tt_loc in range(c.TPC):
                    tt = tcc * c.TPC + tt_loc
                    xt = io.tile([128, c.E], BF16, tag="io")
                    nc.sync.dma_start(xt[:], x_d[tt * 128:(tt + 1) * 128, :])
                    xts.append(xt)

                for tt_loc in range(c.TPC):
                    h = layernorm_tile(xts[tt_loc])
                    transpose_to(h, hTc[:, :], tt_loc)

                # V for this chunk's t-tiles
                for tt_loc in range(c.TPC):
                    tt = tcc * c.TPC + tt_loc
                    vps = ps_acc.tile([128, c.D], F32, tag="acc")
                    for e in range(c.EC):
                        nc.tensor.matmul(
                            vps[:],
                            hTc[:, e * c.QCW + tt_loc * 128:][:, :128],
                            wvt[:, e * c.D:(e + 1) * c.D],
                            start=(e == 0), stop=(e == c.EC - 1),
                        )
                    vdst = v_sb[
                        :, tt * c.HPC * 65:(tt + 1) * c.HPC * 65
                    ].rearrange("p (h w) -> p h w", w=65)
                    nc.vector.tensor_copy(
                        vdst[:, :, 0:64],
                        vps[:].rearrange("p (h w) -> p h w", w=64),
                    )
                    nc.vector.memset(vdst[:, :, 64:65], 1.0)

                # K then Q for this chunk. All kT copies precede all q
                # copies on DVE: a q-pool slot can stall on a previous
                # chunk's attention, and that attention needs this chunk's
                # kT — k-first keeps the queue deadlock-free.
                def kq_pass(w_d, dst_of):
                    # two head-pairs per load: 512B descriptor lines run the
                    # DMA bus at full rate (256B lines pay a 2x latency mult)
                    for p0 in range(0, c.PAIRS, 2):
                        wt2 = wqk_pool.tile(
                            [128, 2 * c.E], BF16, tag="wqk2", bufs=2
                        )
                        nc.gpsimd.dma_start(
                            wt2[:].rearrange("p (e m) -> p e m", e=c.EC),
                            w_d[:, p0 * 128:(p0 + 2) * 128].rearrange(
                                "(e p) m -> p e m", p=128
                            ),
                        )
                        for pp in range(2):
                            p = p0 + pp
                            qk = ps_acc.tile([128, c.QCW], F32, tag="acc")
                            for e in range(c.EC):
                                nc.tensor.matmul(
                                    qk[:],
                                    wt2[
                                        :, e * 256 + pp * 128:
                                        e * 256 + (pp + 1) * 128
                                    ],
                                    hTc[:, e * c.QCW:(e + 1) * c.QCW],
                                    start=(e == 0), stop=(e == c.EC - 1),
                                )
                            nc.vector.tensor_copy(dst_of(p), qk[:])

                kq_pass(wk_d, lambda p: kT(p)[:, r0:r0 + c.QCW])
                qt = qpool.tile([128, c.PAIRS * c.QCW], BF16, tag="q")
                kq_pass(wq_d, lambda p: qt[:, p * c.QCW:(p + 1) * c.QCW])

                return qt

            def phase1b(tcc, wot, qt):
                """Attention + proj -> ar1_in chunk + ReduceScatter."""
                r0 = tcc * c.QCW
                yTc = ypool.tile([128, c.PAIRS * c.QCW], BF16, tag="yT")
                for p in range(c.PAIRS):
                    att_block(p, tcc, yTc[:, :], qt[:])

                for tt_loc in range(c.TPC):
                    tt = tcc * c.TPC + tt_loc
                    pt = pout.tile([128, c.E], BF16, tag="pt")
                    pp = ps_w.tile([128, c.EH * c.NH], F32, tag="w")
                    for eh in range(c.EH):
                        for d in range(c.PAIRS):
                            nc.tensor.matmul(
                                pp[:, eh * c.NH:(eh + 1) * c.NH],
                                yTc[:, d * c.QCW + tt_loc * 128:][:, :128],
                                wot[:, d * c.E + eh * c.NH:][:, :c.NH],
                                start=(d == 0), stop=(d == c.PAIRS - 1),
                            )
                    nc.vector.tensor_copy(pt[:], pp[:])
                    nc.sync.dma_start(ar1_in[tt * 128:(tt + 1) * 128, :], pt[:])

                rows = slice(r0, r0 + c.QCW)
                orows = slice(tcc * 256, (tcc + 1) * 256)
                if c.n_cores == 1 or STUB_CC:  # timeline/profiling variant
                    nc.sync.dma_start(ar1_out[orows, :], ar1_in[rows, :][0:256, :])
                else:
                    nc.gpsimd.collective_compute(
                        "ReduceScatter", mybir.AluOpType.add,
                        replica_groups=pairs_rg,
                        ins=[ar1_in[rows, :]], outs=[ar1_out[orows, :]],
                    )

            def ffn_a(sc, t0=0, t1=None, handles=None):
                """Own-row chunk sc tiles [t0,t1): x2 = xo + rs + bo; LN2 ->
                transposed h2.

                Returns (hTc, x2keep) where x2keep[:, t*E:(t+1)*E] holds the
                bf16 residual (with +b2 folded in) for the epilogue.
                """
                if handles is not None:
                    hTc, x2keep = handles
                elif sc == 0:
                    hTc = hpool.tile([128, c.EC * c.QCW], BF16, tag="hT")
                    x2keep = x2_sb
                else:
                    hTc = hpool.tile([128, c.EC * c.QCW], BF16, tag="hT")
                    x2bt = wv_pool.tile([128, c.TPC * c.E], BF16, tag="wv")
                    x2keep = x2bt[:]
                if t1 is None:
                    t1 = c.TPC

                for tt_loc in range(t0, t1):
                    ro = sc * c.QCW + tt_loc * 128
                    xt = pout.tile([128, c.E], F32, tag="x2")
                    nc.sync.dma_start(xt[:], xo_d[ro:ro + 128, :])
                    at = pout.tile([128, c.E], BF16, tag="x2")
                    nc.sync.dma_start(at[:], ar1_out[ro:ro + 128, :])
                    # xo+bo is ready before the collective lands; keep the
                    # RS-dependent chain to a single add
                    xb = pout.tile([128, c.E], F32, tag="pt")
                    nc.vector.tensor_add(xb[:], xt[:], bo_r[:])
                    x2 = pout.tile([128, c.E], F32, tag="pt")
                    nc.vector.tensor_add(x2[:], xb[:], at[:])
                    # residual copy carries +b2 so the epilogue is one add
                    nc.vector.tensor_add(
                        x2keep[:, tt_loc * c.E:(tt_loc + 1) * c.E],
                        x2[:], b2_r[:],
                    )
                    h2 = layernorm_tile(x2)
                    transpose_to(h2, hTc[:, :], tt_loc)
                return hTc, x2keep

            def w1_load(fg, eng=None):
                w1t = w1_pool.tile([128, 2 * c.E], BF16, tag="w1")
                (eng or nc.gpsimd).dma_start(
                    w1t[:].rearrange("p (e m) -> p e m", e=c.EC),
                    w1_d[:, 2 * fg * 128:(2 * fg + 2) * 128].rearrange(
                        "(e p) m -> p e m", p=128
                    ),
                )
                return w1t

            def w2_load(eh, fg, eng=None):
                t = w2_pool.tile([128, 2 * c.NH], BF16, tag="w2")
                (eng or nc.gpsimd).dma_start(
                    t[:].rearrange("p (g n) -> p g n", g=2),
                    w2_d[
                        2 * fg * 128:(2 * fg + 2) * 128,
                        eh * c.NH:(eh + 1) * c.NH,
                    ].rearrange("(g p) n -> p g n", p=128),
                )
                return t

            def ffn_b1(sc, hTc, w1pre=None, c0=0, c1=None):
                """FFN layer 1 cols [c0,c1): aT[f] = relu(w1_f.T @ h2T + b1)."""
                if c1 is None:
                    c1 = c.QCW
                w = c1 - c0
                # sc0 overlaps attention: keep its w1 stream off the Act
                # queue (softmax exps); sc1 runs post-attention on Act
                w1_eng = nc.gpsimd if sc == 0 else nc.scalar
                nfg = c.FC // 2
                for fg in range(nfg):
                    if w1pre is not None and fg == 0:
                        w1t = w1pre
                    else:
                        w1t = w1_load(fg, w1_eng)
                    for gi in range(2):
                        f = 2 * fg + gi
                        ap_ = ps_acc.tile([128, c.QCW], F32, tag="acc")
                        for e in range(c.EC):
                            nc.tensor.matmul(
                                ap_[:, 0:w],
                                w1t[:, e * 256 + gi * 128:][:, :128],
                                hTc[:, e * c.QCW + c0:e * c.QCW + c1],
                                start=(e == 0), stop=(e == c.EC - 1),
                            )
                        # relu on DVE keeps the Act engine free for the
                        # softmax exps it bottlenecks on
                        nc.vector.tensor_scalar(
                            aT(f)[:, c0:c1], ap_[:, 0:w],
                            b1_sb[:, f:f + 1], 0.0,
                            mybir.AluOpType.add, mybir.AluOpType.max,
                        )

            def ffn_b2(sc, ehs, x2keep, t0=0, t1=None):
                """FFN layer 2 + residual epilogue: out rows = x2 + ff + b2."""
                if t1 is None:
                    t1 = c.TPC
                nfg = c.FC // 2
                for eh in ehs:
                    ffps = []
                    for _fi in range((t1 - t0) // 2):
                        fftile = ps_w.tile([128, 2 * c.NH], F32, tag="w")
                        ffps.append(fftile)
                    w2_eng = nc.gpsimd if sc == 0 else nc.scalar
                    for fg in range(nfg):
                        w2t = w2_load(eh, fg, w2_eng)
                        for gi in range(2):
                            f = 2 * fg + gi
                            for ti in range(t0, t1):
                                nc.tensor.matmul(
                                    ffps[(ti - t0) // 2][
                                        :,
                                        ((ti - t0) % 2) * c.NH:
                                        ((ti - t0) % 2 + 1) * c.NH,
                                    ],
                                    aT(f)[
                                        :,
                                        ti * 128:(ti + 1) * 128,
                                    ],
                                    w2t[:, gi * c.NH:(gi + 1) * c.NH],
                                    start=(f == 0), stop=(f == c.FC - 1),
                                )
                    for wi in range((t1 - t0) // 2):
                        for bi in range(2):
                            ti = t0 + 2 * wi + bi
                            ro = sc * c.QCW + ti * 128
                            ft = ffo.tile([128, c.NH], F32, tag="ffout")
                            nc.vector.tensor_add(
                                ft[:],
                                ffps[wi][:, bi * c.NH:(bi + 1) * c.NH],
                                x2keep[
                                    :, ti * c.E + eh * c.NH:
                                    ti * c.E + (eh + 1) * c.NH
                                ],
                            )
                            nc.scalar.dma_start(
                                out_d[ro:ro + 128, eh * c.NH:(eh + 1) * c.NH],
                                ft[:],
                            )

            # ---- emission schedule ----
            # Natural chunk order; RS(tcc) lands after each chunk's proj.
            # sc0 (chunks 0,1 rows) LN+FFN runs between chunks 2/3 so the
            # tensor engine has queued FFN work while RS3 runs; sc1 splits
            # into per-RS-chunk halves on the post-RS2/RS3 critical path.
            wvt, wot = load_wv_wo()
            qt = phase1a(0, wvt)
            phase1b(0, wot, qt)                       # -> RS0
            qt = phase1a(1, wvt)
            phase1b(1, wot, qt)                       # -> RS1
            qt = phase1a(2, wvt)
            with tc.tile_wait_until(FFN_A0_MS):
                h2a, x2a = ffn_a(0)
            phase1b(2, wot, qt)                       # -> RS2
            qt = phase1a(3, wvt)
            ffn_b1(0, h2a)
            phase1b(3, wot, qt)                       # -> RS3 (last)
            ffn_b2(0, [0], x2a)
            ffn_b2(0, [1], x2a)
            # sc1 first half (chunk-2 rows): ready after RS2
            with tc.tile_wait_until(FFN_A1_MS):
                h2b, x2b = ffn_a(1, 0, 2)
            ffn_b1(1, h2b, c0=0, c1=256)
            ffn_b2(1, [0, 1], x2b, t0=0, t1=2)
            # sc1 second half (chunk-3 rows): ready after RS3
            with tc.tile_wait_until(FFN_A1B_MS):
                ffn_a(1, 2, 4, handles=(h2b, x2b))
            ffn_b1(1, h2b, c0=256, c1=512)
            ffn_b2(1, [0, 1], x2b, t0=2, t1=4)

    nc.compile()
    return nc


def make_masks(cfg):
    """Additive causal masks: 0 where attending is allowed, -30 where not."""
    c = cfg
    m = np.zeros((128, c.KTPQ * c.QCW), dtype=np.float32)
    for j in range(c.KTPQ):
        k = np.arange(128)[:, None]
        q = np.arange(c.QCW)[None, :]
        mj = np.where(j * 128 + k <= q, 0.0, -30.0).astype(np.float32)
        m[:, j * c.QCW:(j + 1) * c.QCW] = mj
    return np.ascontiguousarray(m.astype(ml_dtypes.bfloat16))


def own_rows(cfg, g):
    """Global row indices owned by group-half g after the ReduceScatter."""
    c = cfg
    idx = []
    for tcc in range(c.TC):
        s = tcc * c.QCW + g * 256
        idx.extend(range(s, s + 256))
    return np.array(idx)


def make_in_maps(cfg, inputs):
    """Build the per-core input dicts from the full problem inputs."""
    c = cfg
    x = np.asarray(inputs["x"], dtype=np.float32)
    ln1 = np.asarray(inputs["ln1_w"], dtype=np.float32)
    ln2 = np.asarray(inputs["ln2_w"], dtype=np.float32)
    # fold LN scale vectors into the consumer weight rows
    Wq = (ln1[:, None] * np.asarray(inputs["Wq"], dtype=np.float32)
          * (1.0 / np.sqrt(c.E)))
    Wk = ln1[:, None] * np.asarray(inputs["Wk"], dtype=np.float32)
    Wv = ln1[:, None] * np.asarray(inputs["Wv"], dtype=np.float32)
    W1 = ln2[:, None] * np.asarray(inputs["W1"], dtype=np.float32)
    Wo = np.asarray(inputs["Wo"], dtype=np.float32)
    W2 = np.asarray(inputs["W2"], dtype=np.float32)
    bo = np.asarray(inputs["bo"], dtype=np.float32)
    b1 = np.asarray(inputs["b1"], dtype=np.float32)
    b2 = np.asarray(inputs["b2"], dtype=np.float32)

    def rep(v):
        return np.ascontiguousarray(
            np.broadcast_to(v[None, :], (128, c.E)).astype(ml_dtypes.bfloat16)
        )

    consts = {
        "bor": rep(bo), "b2r": rep(b2),
        "masks": make_masks(c),
        "ident": np.eye(128, dtype=np.float32),
        "identb": np.ascontiguousarray(
            np.eye(128, dtype=np.float32).astype(ml_dtypes.bfloat16)
        ),
        "w1": np.ascontiguousarray(W1.astype(ml_dtypes.bfloat16)),
        "w2": np.ascontiguousarray(W2.astype(ml_dtypes.bfloat16)),
        "b1": np.ascontiguousarray(b1.reshape(c.FC, 128).T),
    }
    in_maps = []
    for core in range(c.n_cores):
        b, g = core // 2, core % 2
        d0, d1 = g * c.D, (g + 1) * c.D
        m = {
            "x": np.ascontiguousarray(x[b].astype(ml_dtypes.bfloat16)),
            "xo": np.ascontiguousarray(x[b][own_rows(c, g)]),
            "wq": np.ascontiguousarray(Wq[:, d0:d1].astype(ml_dtypes.bfloat16)),
            "wk": np.ascontiguousarray(Wk[:, d0:d1].astype(ml_dtypes.bfloat16)),
            "wv": np.ascontiguousarray(Wv[:, d0:d1].astype(ml_dtypes.bfloat16)),
            "wo": np.ascontiguousarray(Wo[d0:d1, :].astype(ml_dtypes.bfloat16)),
        }
        m.update(consts)
        in_maps.append(m)
    return in_maps


_NC_CACHE = {}


def get_nc(cfg):
    key = (cfg.B, cfg.T, cfg.E, cfg.HPC, cfg.FH, cfg.n_cores)
    if key not in _NC_CACHE:
        _NC_CACHE[key] = build_nc(cfg)
    return _NC_CACHE[key]


def assemble_out(cfg, results):
    c = cfg
    out = np.empty((c.B, c.T, c.E), dtype=np.float32)
    for b in range(c.B):
        for g in range(2):
            out[b][own_rows(c, g)] = results[2 * b + g]["out"]
    return out


def kernel(**inputs) -> np.ndarray:
    c = FULL
    nc = get_nc(c)
    in_maps = make_in_maps(c, inputs)
    res = run_bass_kernel_spmd(nc, in_maps, core_ids=list(range(c.n_cores)))
    return assemble_out(c, res.results)
